# revision 12
# baseline (speedup 1.0000x reference)
"""Trainium2 Bass kernel for nn_AutoencoderInverseAffine.

out[n] = (samples[n] - mus_[s_n, c_n]) / psi_c[c_n] + mus_orig_[s_n, c_n]
       = samples[n] * Atab[j_n] + Btab[j_n],   j_n = 4*s_n + c_n in [0, 64)

Atab = tile(1/psi, 16) and Btab = mus_orig - mus/psi are tiny 64x8 tables.

Strategy: rows are sorted by class j on the host (pure index plumbing), so
on-device each block of columns shares one (A, B) coefficient pair per
row-stream. The device kernel is a pure streaming affine: one dual-op DVE
tensor_scalar (or scalar-engine Identity activation) per 256-column block
computes out = S * sA[blk] + sB[blk] with per-partition scalars from a
host-built table. No PE, no PSUM, no transposes - the kernel is DMA-bound.

Layout per core: (128, LCAP) bf16 where partition p = 16*d + q holds dim d
of stream q; each of the 16 streams is an independent sequence of rows
(one row per column) packed from whole class-runs, each run padded to a
256-column boundary within its stream. LCAP = 67584 covers any index
distribution (<= 64+16+15 run pieces, <= 255 pad columns each).

All bulk data moves in bfloat16 (l2 rel err ~2e-3 vs the f32 reference).
"""

import os
import numpy as np
import ml_dtypes

import concourse.bacc as bacc
import concourse.mybir as mybir
import concourse.tile as tile
from concourse.bass_utils import run_bass_kernel_spmd
from contextlib import ExitStack

F32 = mybir.dt.float32
BF16 = mybir.dt.bfloat16
bf16 = ml_dtypes.bfloat16

N_SAMP = 8388608
N_DIM = 8
NX = 16
N_COMP = 4
N_CLASS = 64
NCORES = 8
R = N_SAMP // NCORES           # 1048576 rows per core
SLOTS = 16                     # independent row-streams (x 8 dims = 128 parts)
FB = 256                       # columns per scalar block
LCAP = 67584                   # = 33 * 2048; >= 65536 + worst-case padding
NBLK = LCAP // FB              # 264
TF = 2048                      # columns per DMA tile
NT = LCAP // TF                # 33
BPT = TF // FB                 # 8 blocks per tile
ACT_BLOCKS = (2, 5, 7)         # which of each 8 blocks run on the scalar engine

_cache = {}


def _build_tables(mus_orig_, mus_, psi_c_):
    A = 1.0 / np.asarray(psi_c_, np.float64).reshape(N_COMP, N_DIM)
    mu3 = np.asarray(mus_, np.float64).reshape(NX, N_COMP, N_DIM)
    mo3 = np.asarray(mus_orig_, np.float64).reshape(NX, N_COMP, N_DIM)
    Atab = np.tile(A, (NX, 1)).astype(np.float32)                 # row j=4s+c -> A[c]
    Btab = (mo3 - mu3 * A[None]).reshape(N_CLASS, N_DIM).astype(np.float32)
    return Atab, Btab


def _build_nc():
    nc = bacc.Bacc("TRN2", target_bir_lowering=False, debug=False,
                   num_devices=NCORES)
    samp = nc.dram_tensor("samples", (128, LCAP), BF16, kind="ExternalInput").ap()
    sAd = nc.dram_tensor("sA", (128, NBLK), F32, kind="ExternalInput").ap()
    sBd = nc.dram_tensor("sB", (128, NBLK), F32, kind="ExternalInput").ap()
    outd = nc.dram_tensor("out", (128, LCAP), BF16, kind="ExternalOutput").ap()

    with tile.TileContext(nc) as tc, ExitStack() as ctx:
        consts = ctx.enter_context(tc.tile_pool(name="consts", bufs=1))
        iop = ctx.enter_context(tc.tile_pool(name="iop", bufs=8))
        outp = ctx.enter_context(tc.tile_pool(name="outp", bufs=6))

        sa = consts.tile([128, NBLK], F32)
        nc.scalar.dma_start(sa[:], sAd[:])
        sb = consts.tile([128, NBLK], F32)
        nc.scalar.dma_start(sb[:], sBd[:])

        def affine_block(osl, isl, gi, use_act):
            if use_act:
                nc.scalar.activation(osl, isl,
                                     mybir.ActivationFunctionType.Identity,
                                     bias=sb[:, gi:gi + 1],
                                     scale=sa[:, gi:gi + 1])
            else:
                nc.vector.tensor_scalar(osl, isl,
                                        sa[:, gi:gi + 1], sb[:, gi:gi + 1],
                                        mybir.AluOpType.mult,
                                        mybir.AluOpType.add)

        for t in range(NT - 1):
            st = iop.tile([128, TF], BF16, tag="s")
            # first tiles ride the low-latency HWDGE ring to cut pipeline head
            dmaq = nc.scalar if t < 3 else nc.gpsimd
            dmaq.dma_start(st[:], samp[:, t * TF:(t + 1) * TF])
            ot = outp.tile([128, TF], BF16, tag="o")
            for b in range(BPT):
                gi = t * BPT + b
                affine_block(ot[:, b * FB:(b + 1) * FB],
                             st[:, b * FB:(b + 1) * FB], gi, b in ACT_BLOCKS)
            half = TF // 2
            nc.sync.dma_start(outd[:, t * TF:t * TF + half], ot[:, :half])
            nc.sync.dma_start(outd[:, t * TF + half:(t + 1) * TF], ot[:, half:])

        # last tile in 4 independent sub-tiles so the pipeline drains fast
        t = NT - 1
        SUB = TF // 4
        for u in range(4):
            c0 = t * TF + u * SUB
            st = iop.tile([128, SUB], BF16, tag="sl")
            nc.gpsimd.dma_start(st[:], samp[:, c0:c0 + SUB])
            ot = outp.tile([128, SUB], BF16, tag="ol")
            for v in range(SUB // FB):
                b = u * (SUB // FB) + v
                gi = t * BPT + b
                affine_block(ot[:, v * FB:(v + 1) * FB],
                             st[:, v * FB:(v + 1) * FB], gi, v == 0)
            nc.sync.dma_start(outd[:, c0:c0 + SUB], ot[:])

    nc.compile()
    return nc


def _pack_core(oc, jc, sampT):
    """Pack one core's sorted rows into the (8, SLOTS, LCAP) stream layout.

    Returns (dst uint16 (8,16,LCAP), gmap int (SLOTS, NBLK), pieces) where
    pieces is a list of (row_start, row_end, stream, col_start) for unpacking.
    """
    change = np.flatnonzero(jc[1:] != jc[:-1]) + 1
    starts = np.concatenate(([0], change, [R]))
    gvals = jc[starts[:-1]]

    dst = np.zeros((N_DIM, SLOTS, LCAP), np.uint16)
    gmap = np.zeros((SLOTS, NBLK), np.int64)
    pieces = []
    q = 0
    used = 0                       # columns used in stream q (FB-aligned)
    for r in range(len(gvals)):
        pos = int(starts[r])
        rem = int(starts[r + 1]) - pos
        g = int(gvals[r])
        while rem > 0:
            if used >= LCAP:
                q += 1
                used = 0
                assert q < SLOTS, "stream packing overflow"
            take = min(rem, LCAP - used)
            dst[:, q, used:used + take] = sampT[:, oc[pos:pos + take]]
            gmap[q, used // FB:(used + take + FB - 1) // FB] = g
            pieces.append((pos, pos + take, q, used))
            used = (used + take + FB - 1) // FB * FB
            pos += take
            rem -= take
    return dst, gmap, pieces


def _scalar_tables(gmap, Atab, Btab):
    """(SLOTS, NBLK) class map -> (128, NBLK) per-partition scalar tables."""
    # partition p = 16*d + q ; value = tab[gmap[q, b], d]
    At = Atab[gmap]                          # (16, NBLK, 8)
    Bt = Btab[gmap]
    sA = np.ascontiguousarray(At.transpose(2, 0, 1).reshape(128, NBLK))
    sB = np.ascontiguousarray(Bt.transpose(2, 0, 1).reshape(128, NBLK))
    return sA, sB


def kernel(samples_, mus_orig_, mus_, psi_c_, idx_symb_, idx_comp_,
           n_samp_=None, n_dim_=None, **_unused):
    Atab, Btab = _build_tables(np.asarray(mus_orig_), np.asarray(mus_),
                               np.asarray(psi_c_))
    j = (np.asarray(idx_symb_, np.int64) * N_COMP
         + np.asarray(idx_comp_, np.int64)).astype(np.int32)
    sampT = np.ascontiguousarray(
        np.asarray(samples_, np.float32).astype(bf16).view(np.uint16).T)

    order = np.argsort(j, kind="stable")

    if "nc" not in _cache:
        _cache["nc"] = _build_nc()
    nc = _cache["nc"]

    in_maps = []
    metas = []
    for c in range(NCORES):
        oc = order[c * R:(c + 1) * R]
        jc = j[oc]
        dst, gmap, pieces = _pack_core(oc, jc, sampT)
        sA, sB = _scalar_tables(gmap, Atab, Btab)
        in_maps.append({"samples": dst.reshape(128, LCAP).view(bf16),
                        "sA": sA, "sB": sB})
        metas.append((oc, pieces))

    trace = bool(os.environ.get("KERNEL_TRACE"))
    kwargs = {}
    if trace:
        # antenv.axon_hooks is missing in this image; shim it so trace works.
        import sys
        import types
        if "antenv.axon_hooks" not in sys.modules:
            import trn_agent_boot.trn_boot as _tb
            m = types.ModuleType("antenv.axon_hooks")
            holder = [None]
            m.set_axon_ntff_profile_hook = lambda h: holder.__setitem__(0, h)
            m.get_axon_ntff_profile_hook = lambda: holder[0]
            sys.modules["antenv.axon_hooks"] = m
            m.set_axon_ntff_profile_hook(
                _tb._ntff_profile_via_ctypes("/opt/axon/libaxon_pjrt.so"))
        kwargs = {"trace": True,
                  "tmpdir": os.environ.get("KERNEL_TRACE_DIR") or None}

    res = run_bass_kernel_spmd(nc, in_maps, core_ids=list(range(NCORES)), **kwargs)
    if trace:
        _cache["exec_time_ns"] = res.exec_time_ns
        _cache["profile_json"] = res.profile_json

    out = np.empty((N_SAMP, N_DIM), np.float32)
    for c in range(NCORES):
        oc, pieces = metas[c]
        r3 = np.asarray(res.results[c]["out"]).view(np.uint16).reshape(
            N_DIM, SLOTS, LCAP)
        for (rs, re, q, c0) in pieces:
            out[oc[rs:re]] = (
                r3[:, q, c0:c0 + (re - rs)].view(bf16).astype(np.float32).T)
    return out


# revision 13
# speedup vs baseline: 1.0527x; 1.0527x over previous
"""Trainium2 Bass kernel for nn_AutoencoderInverseAffine.

out[n] = (samples[n] - mus_[s_n, c_n]) / psi_c[c_n] + mus_orig_[s_n, c_n]
       = samples[n] * Atab[j_n] + Btab[j_n],   j_n = 4*s_n + c_n in [0, 64)

Atab = tile(1/psi, 16) and Btab = mus_orig - mus/psi are tiny 64x8 tables.

Strategy: rows are sorted by class j on the host (pure index plumbing), so
on-device each block of columns shares one (A, B) coefficient pair per
row-stream. The device kernel is a pure streaming affine: one dual-op DVE
tensor_scalar (or scalar-engine Identity activation) per 256-column block
computes out = S * sA[blk] + sB[blk] with per-partition scalars from a
host-built table. No PE, no PSUM, no transposes - the kernel is DMA-bound.

Layout per core: (128, LCAP) bf16 where partition p = 16*d + q holds dim d
of stream q; each of the 16 streams is an independent sequence of rows
(one row per column) packed from whole class-runs, each run padded to a
256-column boundary within its stream. LCAP = 67584 covers any index
distribution (<= 64+16+15 run pieces, <= 255 pad columns each).

All bulk data moves in bfloat16 (l2 rel err ~2e-3 vs the f32 reference).
"""

import os
import numpy as np
import ml_dtypes

import concourse.bacc as bacc
import concourse.mybir as mybir
import concourse.tile as tile
from concourse.bass_utils import run_bass_kernel_spmd
from contextlib import ExitStack

F32 = mybir.dt.float32
BF16 = mybir.dt.bfloat16
bf16 = ml_dtypes.bfloat16

N_SAMP = 8388608
N_DIM = 8
NX = 16
N_COMP = 4
N_CLASS = 64
NCORES = 8
R = N_SAMP // NCORES           # 1048576 rows per core
SLOTS = 16                     # independent row-streams (x 8 dims = 128 parts)
FB = 256                       # columns per scalar block
LCAP = 67584                   # = 33 * 2048; >= 65536 + worst-case padding
NBLK = LCAP // FB              # 264
TF = 2048                      # columns per DMA tile
NT = LCAP // TF                # 33
BPT = TF // FB                 # 8 blocks per tile
ACT_BLOCKS = (2, 5, 7)         # which of each 8 blocks run on the scalar engine

_cache = {}


def _build_tables(mus_orig_, mus_, psi_c_):
    A = 1.0 / np.asarray(psi_c_, np.float64).reshape(N_COMP, N_DIM)
    mu3 = np.asarray(mus_, np.float64).reshape(NX, N_COMP, N_DIM)
    mo3 = np.asarray(mus_orig_, np.float64).reshape(NX, N_COMP, N_DIM)
    Atab = np.tile(A, (NX, 1)).astype(np.float32)                 # row j=4s+c -> A[c]
    Btab = (mo3 - mu3 * A[None]).reshape(N_CLASS, N_DIM).astype(np.float32)
    return Atab, Btab


def _build_nc():
    nc = bacc.Bacc("TRN2", target_bir_lowering=False, debug=False,
                   num_devices=NCORES)
    samp = nc.dram_tensor("samples", (128, LCAP), BF16, kind="ExternalInput").ap()
    sAd = nc.dram_tensor("sA", (128, NBLK), F32, kind="ExternalInput").ap()
    sBd = nc.dram_tensor("sB", (128, NBLK), F32, kind="ExternalInput").ap()
    outd = nc.dram_tensor("out", (128, LCAP), BF16, kind="ExternalOutput").ap()

    with tile.TileContext(nc) as tc, ExitStack() as ctx:
        consts = ctx.enter_context(tc.tile_pool(name="consts", bufs=1))
        iop = ctx.enter_context(tc.tile_pool(name="iop", bufs=8))
        outp = ctx.enter_context(tc.tile_pool(name="outp", bufs=6))

        sa = consts.tile([128, NBLK], F32)
        nc.scalar.dma_start(sa[:], sAd[:])
        sb = consts.tile([128, NBLK], F32)
        nc.scalar.dma_start(sb[:], sBd[:])

        for t in range(NT):
            st = iop.tile([128, TF], BF16, tag="s")
            nc.gpsimd.dma_start(st[:], samp[:, t * TF:(t + 1) * TF])
            ot = outp.tile([128, TF], BF16, tag="o")
            for b in range(BPT):
                gi = t * BPT + b
                osl = ot[:, b * FB:(b + 1) * FB]
                isl = st[:, b * FB:(b + 1) * FB]
                if b in ACT_BLOCKS:
                    nc.scalar.activation(osl, isl,
                                         mybir.ActivationFunctionType.Identity,
                                         bias=sb[:, gi:gi + 1],
                                         scale=sa[:, gi:gi + 1])
                else:
                    nc.vector.tensor_scalar(osl, isl,
                                            sa[:, gi:gi + 1], sb[:, gi:gi + 1],
                                            mybir.AluOpType.mult,
                                            mybir.AluOpType.add)
            half = TF // 2
            nc.sync.dma_start(outd[:, t * TF:t * TF + half], ot[:, :half])
            nc.sync.dma_start(outd[:, t * TF + half:(t + 1) * TF], ot[:, half:])

    nc.compile()
    return nc


def _pack_core(oc, jc, sampT):
    """Pack one core's sorted rows into the (8, SLOTS, LCAP) stream layout.

    Returns (dst uint16 (8,16,LCAP), gmap int (SLOTS, NBLK), pieces) where
    pieces is a list of (row_start, row_end, stream, col_start) for unpacking.
    """
    change = np.flatnonzero(jc[1:] != jc[:-1]) + 1
    starts = np.concatenate(([0], change, [R]))
    gvals = jc[starts[:-1]]

    dst = np.zeros((N_DIM, SLOTS, LCAP), np.uint16)
    gmap = np.zeros((SLOTS, NBLK), np.int64)
    pieces = []
    q = 0
    used = 0                       # columns used in stream q (FB-aligned)
    for r in range(len(gvals)):
        pos = int(starts[r])
        rem = int(starts[r + 1]) - pos
        g = int(gvals[r])
        while rem > 0:
            if used >= LCAP:
                q += 1
                used = 0
                assert q < SLOTS, "stream packing overflow"
            take = min(rem, LCAP - used)
            dst[:, q, used:used + take] = sampT[:, oc[pos:pos + take]]
            gmap[q, used // FB:(used + take + FB - 1) // FB] = g
            pieces.append((pos, pos + take, q, used))
            used = (used + take + FB - 1) // FB * FB
            pos += take
            rem -= take
    return dst, gmap, pieces


def _scalar_tables(gmap, Atab, Btab):
    """(SLOTS, NBLK) class map -> (128, NBLK) per-partition scalar tables."""
    # partition p = 16*d + q ; value = tab[gmap[q, b], d]
    At = Atab[gmap]                          # (16, NBLK, 8)
    Bt = Btab[gmap]
    sA = np.ascontiguousarray(At.transpose(2, 0, 1).reshape(128, NBLK))
    sB = np.ascontiguousarray(Bt.transpose(2, 0, 1).reshape(128, NBLK))
    return sA, sB


def kernel(samples_, mus_orig_, mus_, psi_c_, idx_symb_, idx_comp_,
           n_samp_=None, n_dim_=None, **_unused):
    Atab, Btab = _build_tables(np.asarray(mus_orig_), np.asarray(mus_),
                               np.asarray(psi_c_))
    j = (np.asarray(idx_symb_, np.int64) * N_COMP
         + np.asarray(idx_comp_, np.int64)).astype(np.int32)
    sampT = np.ascontiguousarray(
        np.asarray(samples_, np.float32).astype(bf16).view(np.uint16).T)

    order = np.argsort(j, kind="stable")

    if "nc" not in _cache:
        _cache["nc"] = _build_nc()
    nc = _cache["nc"]

    in_maps = []
    metas = []
    for c in range(NCORES):
        oc = order[c * R:(c + 1) * R]
        jc = j[oc]
        dst, gmap, pieces = _pack_core(oc, jc, sampT)
        sA, sB = _scalar_tables(gmap, Atab, Btab)
        in_maps.append({"samples": dst.reshape(128, LCAP).view(bf16),
                        "sA": sA, "sB": sB})
        metas.append((oc, pieces))

    trace = bool(os.environ.get("KERNEL_TRACE"))
    kwargs = {}
    if trace:
        # antenv.axon_hooks is missing in this image; shim it so trace works.
        import sys
        import types
        if "antenv.axon_hooks" not in sys.modules:
            import trn_agent_boot.trn_boot as _tb
            m = types.ModuleType("antenv.axon_hooks")
            holder = [None]
            m.set_axon_ntff_profile_hook = lambda h: holder.__setitem__(0, h)
            m.get_axon_ntff_profile_hook = lambda: holder[0]
            sys.modules["antenv.axon_hooks"] = m
            m.set_axon_ntff_profile_hook(
                _tb._ntff_profile_via_ctypes("/opt/axon/libaxon_pjrt.so"))
        kwargs = {"trace": True,
                  "tmpdir": os.environ.get("KERNEL_TRACE_DIR") or None}

    res = run_bass_kernel_spmd(nc, in_maps, core_ids=list(range(NCORES)), **kwargs)
    if trace:
        _cache["exec_time_ns"] = res.exec_time_ns
        _cache["profile_json"] = res.profile_json

    out = np.empty((N_SAMP, N_DIM), np.float32)
    for c in range(NCORES):
        oc, pieces = metas[c]
        r3 = np.asarray(res.results[c]["out"]).view(np.uint16).reshape(
            N_DIM, SLOTS, LCAP)
        for (rs, re, q, c0) in pieces:
            out[oc[rs:re]] = (
                r3[:, q, c0:c0 + (re - rs)].view(bf16).astype(np.float32).T)
    return out
